# revision 55
# baseline (speedup 1.0000x reference)
"""Masked-attention Trainium2 kernel, SPMD over 8 NeuronCores.

Sharding: 8 cores = batch (4) x head-group (2 groups of 6 heads).
Host-side: gather unmasked tokens per sample (mask zeroes both the token
inputs and gates attention, so masked tokens drop out entirely), run dense
attention over the gathered tokens on-device, scatter back, and sum the two
head-group partial projections per sample.
"""

import numpy as np
import ml_dtypes

import concourse.bass as bass
from concourse import bacc
import concourse.mybir as mybir
import concourse.tile as tile
from concourse.bass_utils import run_bass_kernel_spmd

B, N, C, H = 4, 2048, 768, 12
D = C // H              # 64
G = 2                   # head groups
HG = H // G             # 6 heads per core
DH = HG * D             # 384 qkv rows per group
CT = C // 128           # 6 contraction tiles
PT = DH // 128          # 3
NEG = -65504.0
SCALE = D ** -0.5

F32 = mybir.dt.float32
BF16 = mybir.dt.bfloat16
EXP = mybir.ActivationFunctionType.Exp

bf16 = ml_dtypes.bfloat16


def _chunks(total, size):
    out = []
    o = 0
    while o < total:
        s = min(size, total - o)
        out.append((o, s))
        o += s
    return out


def build_nc(Mp: int, Mq: int) -> bass.Bass:
    """Per-core SPMD program.

    Mp: padded gathered-token count (multiple of 128) — key/value range.
    Mq: real max token count over cores — query range actually computed.
    """
    KT = Mp // 128      # key tiles (also output row tiles)
    nc = bacc.Bacc()

    # packed [xgT | wqT | wkT | wvT] along the free dim
    XW = Mp + 3 * DH
    xw_d = nc.declare_dram_parameter("xw", [C, XW], BF16, isOutput=False)
    wpT_d = nc.declare_dram_parameter("wpT", [DH, C], BF16, isOutput=False)
    kb_d = nc.declare_dram_parameter("kbias", [128, KT], F32, isOutput=False)
    y_d = nc.declare_dram_parameter("y", [Mp, C], BF16, isOutput=True)

    # query chunks: a leading remainder then 512-wide chunks, so the
    # 512-wide ones pack both heads' S tiles into one [128,1024] PSUM
    # tile with a single exp per kt
    if Mq > 576:
        n = (Mq - 65) // 512          # fewest 512-chunks with leader <= 576
        r = Mq - 512 * n
        qchunks = [(0, r)] + [(r + 512 * i, 512) for i in range(n)]
    else:
        qchunks = [(0, Mq)]

    with tile.TileContext(nc, pool_alloc_mode="queue") as tc:
        with (
            tc.tile_pool(name="persist", bufs=1) as pp,
            tc.tile_pool(name="work", bufs=3) as wp,
            tc.tile_pool(name="small", bufs=4) as sp,
        ):
            # ---- load inputs: one DMA per contraction tile ----
            xw = pp.tile([128, CT, XW], BF16, tag="xw")
            xw_r = xw_d.rearrange("(ct p) m -> p ct m", p=128)
            for kt in range(CT):
                eng = nc.sync if kt % 2 == 0 else nc.scalar
                eng.dma_start(out=xw[:, kt, :], in_=xw_r[:, kt, :])
            xgT = xw[:, :, 0:Mp]
            wq = xw[:, :, Mp:Mp + DH]
            wk = xw[:, :, Mp + DH:Mp + 2 * DH]
            wv = xw[:, :, Mp + 2 * DH:Mp + 3 * DH]
            wpt = [pp.tile([128, C], BF16, name=f"wp{t}", tag=f"wp{t}")
                   for t in range(PT)]
            for t in range(PT):
                nc.sync.dma_start(out=wpt[t], in_=wpT_d[t * 128:(t + 1) * 128, :])
            kb = pp.tile([128, KT], F32, tag="kb")
            nc.sync.dma_start(out=kb, in_=kb_d[:, :])
            # warm ACT with a kb read so exp never stalls on the kb DMA
            actw = sp.tile([1, KT], F32, tag="actw")
            nc.scalar.copy(out=actw, in_=kb[0:1, :])

            # ---- persistent intermediates ----
            # per-chunk q tiles and per-key-block k tiles for fine deps
            qcks = _chunks(Mq, 512)
            qT = [[pp.tile([128, cs], BF16, name=f"qT{t}_{i}", tag=f"qT{t}_{i}")
                   for i, (c0, cs) in enumerate(qcks)] for t in range(PT)]
            kcks = _chunks(Mp, 512)
            kT = [[pp.tile([128, cs], BF16, name=f"kT{t}_{i}", tag=f"kT{t}_{i}")
                   for i, (c0, cs) in enumerate(kcks)] for t in range(PT)]
            # even heads: [v | ones] (65 cols, out partitions 0..64)
            # odd heads: [ones | 0..0 | v] (128 cols, out partitions 64..127,
            # denominator at partition 0) so head pairs stack into one
            # 128-partition output tile and the projection contracts K=128
            vse = [pp.tile([128, PT, D + 1], BF16, name=f"ve{m}", tag=f"ve{m}")
                   for m in range(KT)]
            vso = [pp.tile([128, PT, 128], BF16, name=f"vo{m}", tag=f"vo{m}")
                   for m in range(KT)]
            outT = [pp.tile([128, Mp], BF16, name=f"o{t}", tag=f"o{t}")
                    for t in range(PT)]
            ybuf = pp.tile([128, KT, C], BF16, tag="ybuf")
            ones64 = pp.tile([128, 64], F32, tag="ones64")
            nc.vector.memset(ones64, 1.0)
            if Mq < Mp:
                for t in range(PT):
                    nc.vector.memset(outT[t][:, Mq:Mp], 0.0)

            with tc.tile_pool(name="ps", bufs=2, space="PSUM") as psp:
                # ---- phase A group emitters ----
                def emit_qk(t, c0, cs, w_sb, which):
                    ps = psp.tile([128, cs], F32, tag="ops", name="qk")
                    for kt in range(CT):
                        nc.tensor.matmul(
                            ps,
                            lhsT=w_sb[:, kt, t * 128:(t + 1) * 128],
                            rhs=xgT[:, kt, c0:c0 + cs],
                            start=(kt == 0), stop=(kt == CT - 1))
                    if which == "q":
                        qi = [i for i, (o, s) in enumerate(qcks) if o == c0][0]
                        nc.vector.tensor_copy(out=qT[t][qi], in_=ps)
                    else:
                        # scatter into the 512-wide key-chunk tiles
                        o = c0
                        while o < c0 + cs:
                            ki = o // 512
                            lo = o - ki * 512
                            take = min(kcks[ki][1] - lo, c0 + cs - o)
                            nc.vector.tensor_copy(
                                out=kT[t][ki][:, lo:lo + take],
                                in_=ps[:, o - c0:o - c0 + take])
                            o += take

                def emit_v(m):
                    vps = psp.tile([128, DH], F32, tag="ops", name="vps")
                    for kt in range(CT):
                        nc.tensor.matmul(
                            vps,
                            lhsT=xgT[:, kt, m * 128:(m + 1) * 128],
                            rhs=wv[:, kt, :],
                            start=(kt == 0), stop=(kt == CT - 1))
                    vv = vps.rearrange("p (t x d) -> p t x d", x=2, d=D)
                    nc.vector.tensor_copy(out=vse[m][:, :, 0:D], in_=vv[:, :, 0, :])
                    nc.vector.memset(vse[m][:, :, D:D + 1], 1.0)
                    nc.vector.tensor_copy(out=vso[m][:, :, D:2 * D],
                                          in_=vv[:, :, 1, :])
                    nc.vector.memset(vso[m][:, :, 0:1], 1.0)
                    nc.vector.memset(vso[m][:, :, 1:D], 0.0)

                def emit_proj(qt, tail=False):
                    yps = psp.tile([128, C], F32, tag="ops", name="yps")
                    for (n0, ns) in _chunks(C, 512):
                        for t in range(PT):
                            nc.tensor.matmul(
                                yps[:, n0:n0 + ns],
                                lhsT=outT[t][:, qt * 128:(qt + 1) * 128],
                                rhs=wpt[t][:, n0:n0 + ns],
                                start=(t == 0), stop=(t == PT - 1))
                    if tail:
                        nc.scalar.copy(out=ybuf[:, qt, :], in_=yps)
                    else:
                        nc.vector.tensor_copy(out=ybuf[:, qt, :], in_=yps)
                    nc.sync.dma_start(
                        out=y_d[qt * 128:(qt + 1) * 128, :], in_=ybuf[:, qt, :])

                def qk_jobs(t):
                    jobs = []
                    for (c0, cs) in qcks:
                        jobs.append(lambda t=t, c0=c0, cs=cs: emit_qk(t, c0, cs, wq, "q"))
                        jobs.append(lambda t=t, c0=c0, cs=cs: emit_qk(t, c0, cs, wk, "k"))
                    if Mq < Mp:
                        jobs.append(
                            lambda t=t: emit_qk(t, Mq, Mp - Mq, wk, "k"))
                    return jobs

                def norm_head(t, odd, q0, qs, ops, tail=False):
                    dp = 0 if odd else 64
                    r = odd * 64
                    den = sp.tile([65, qs], F32, tag="den")
                    nc.vector.reciprocal(
                        out=den[dp:dp + 1, :], in_=ops[dp:dp + 1, :])
                    if tail:
                        # tail: PE is idle — broadcast via K=1 matmul; stage
                        # the ops rows to SBUF so the mul reads one PSUM
                        osb = sp.tile([128, qs], F32, tag="osb")
                        nc.vector.tensor_copy(
                            out=osb[r:r + 64, :], in_=ops[r:r + 64, :])
                        rbp = psp.tile([128, qs], F32, tag="sps", name="rbp")
                        for (s0, ss) in _chunks(qs, 512):
                            nc.tensor.matmul(
                                rbp[r:r + 64, s0:s0 + ss],
                                lhsT=ones64[dp:dp + 1, :],
                                rhs=den[dp:dp + 1, s0:s0 + ss],
                                start=True, stop=True)
                        nc.vector.tensor_mul(
                            out=outT[t][r:r + 64, q0:q0 + qs],
                            in0=osb[r:r + 64, :], in1=rbp[r:r + 64, :])
                        return
                    rb = sp.tile([128, qs], F32, tag="rb")
                    row = den[dp:dp + 1, :]
                    rep = bass.AP(
                        tensor=row.tensor, offset=row.offset,
                        ap=[list(row.ap[0]), [0, 64], list(row.ap[-1])])
                    nc.gpsimd.dma_start(out=rb[r:r + 64, :], in_=rep)
                    nc.vector.tensor_mul(
                        out=outT[t][r:r + 64, q0:q0 + qs],
                        in0=ops[r:r + 64, :], in1=rb[r:r + 64, :])

                # head 0/1's qT/kT upfront; all other phase-A work is
                # injected into the idle "ops" PSUM slots during kt loops.
                # Constraints: v before block (0,0)'s PV burst; qk(1) before
                # pair 1; qk(2) before pair 2.
                for j in qk_jobs(0):
                    j()
                NC_ = len(qchunks)
                inject = {}
                inject[(0, 0)] = [lambda m=m: emit_v(m) for m in range(KT)]
                if NC_ > 1:
                    inject[(0, 1)] = qk_jobs(1)
                    t2j = qk_jobs(2)
                    inject[(1, 0)] = t2j[:4]
                    inject[(1, 1)] = t2j[4:]
                    nproj = min(4, (qchunks[-1][0]) // 128)
                    inject[(2, NC_ - 1)] = [
                        lambda qt=qt: emit_proj(qt) for qt in range(nproj)]
                else:
                    nproj = 0
                    inject[(0, 0)] += qk_jobs(1)
                    inject[(1, 0)] = qk_jobs(2)

                # ---- attention: pair-major blocks, deferred PV burst ----
                # The PV burst + norms of each block are emitted after the
                # NEXT block's first two S/exp iterations, so ACT keeps
                # streaming exps while PE drains the burst.
                pending = []     # closures: previous block's PV burst + norms

                def s_exp_iter(t, q0, qs, kt):
                    # sub-chunks aligned to the 512-wide q tiles
                    subs = []
                    o = q0
                    while o < q0 + qs:
                        take = min(512 - o % 512, q0 + qs - o)
                        subs.append((o, take))
                        o += take
                    if qs == 512:
                        # both heads' S in one 2-bank tile, one exp call
                        sq2 = psp.tile([128, 2, 512], F32, tag="sps",
                                       name="sq2")
                        kti, klo = divmod(kt * 128, 512)
                        for (a0, ss) in subs:
                            qi = a0 // 512
                            lo = a0 - qi * 512
                            for i, r in ((0, 0), (1, 64)):
                                nc.tensor.matmul(
                                    sq2[:, i, a0 - q0:a0 - q0 + ss],
                                    lhsT=kT[t][kti][r:r + 64, klo:klo + 128],
                                    rhs=qT[t][qi][r:r + 64, lo:lo + ss],
                                    start=True, stop=True)
                        et2 = wp.tile([128, 2, 512], BF16, tag="et",
                                      name="et", bufs=22)
                        nc.scalar.activation(
                            out=et2, in_=sq2, func=EXP,
                            bias=kb[:, kt:kt + 1], scale=1.0)
                        return [et2[:, 0, :], et2[:, 1, :]]
                    sq = [psp.tile([128, qs], F32, tag="sps",
                                   name=f"sps{i}") for i in range(2)]
                    kti, klo = divmod(kt * 128, 512)
                    for (a0, ss) in subs:
                        qi = a0 // 512
                        lo = a0 - qi * 512
                        for i, r in ((0, 0), (1, 64)):
                            nc.tensor.matmul(
                                sq[i][:, a0 - q0:a0 - q0 + ss],
                                lhsT=kT[t][kti][r:r + 64, klo:klo + 128],
                                rhs=qT[t][qi][r:r + 64, lo:lo + ss],
                                start=True, stop=True)
                    pair_ets = []
                    for i in range(2):
                        et = wp.tile([128, qs], BF16, tag="et",
                                     name="et", bufs=22)
                        nc.scalar.activation(
                            out=et, in_=sq[i], func=EXP,
                            bias=kb[:, kt:kt + 1], scale=1.0)
                        pair_ets.append(et)
                    return pair_ets

                for hp in range(PT):
                    t = hp
                    for ci, (q0, qs) in enumerate(qchunks):
                        inj = inject.pop((hp, ci), [])
                        ets = []
                        HOIST = 6 if pending else 0
                        for kt in range(HOIST):
                            ets.append(s_exp_iter(t, q0, qs, kt))
                        for j in pending:
                            j()
                        pending = []
                        for kt in range(HOIST, KT):
                            ets.append(s_exp_iter(t, q0, qs, kt))
                            if kt - HOIST < len(inj):
                                inj[kt - HOIST]()
                        for j in inj[KT - HOIST:]:
                            j()

                        def burst(t=t, q0=q0, qs=qs, ets=ets, tail=False):
                            for odd in (0, 1):
                                opp = 128 if odd else 65
                                lv = vso if odd else vse
                                ops = psp.tile([opp, qs], F32, tag="ops",
                                               name="ops")
                                for kt in range(KT):
                                    for (s0, ss) in _chunks(qs, 512):
                                        nc.tensor.matmul(
                                            ops[:, s0:s0 + ss],
                                            lhsT=lv[kt][:, t, :],
                                            rhs=ets[kt][odd][:, s0:s0 + ss],
                                            start=(kt == 0),
                                            stop=(kt == KT - 1))
                                norm_head(t, odd, q0, qs, ops, tail=tail)
                        pending = [burst]
                for j in pending:
                    j(tail=True)

                # ---- projection (first nproj injected above) ----
                for qt in range(nproj, KT):
                    emit_proj(qt, tail=True)

    nc.finalize()
    return nc


def _prep_core_inputs(x, mask, w_qkv, w_proj, Mp):
    """Returns in_maps (list of 8 dicts) + per-sample gather indices."""
    m = mask[:, :, 0]
    idxs = [np.nonzero(m[b])[0] for b in range(B)]
    in_maps = []
    for core in range(8):
        b, g = divmod(core, G)
        idx = idxs[b]
        M = len(idx)
        xg = np.zeros((Mp, C), np.float32)
        xg[:M] = x[b][idx]
        wq = w_qkv[g * DH:(g + 1) * DH] * SCALE
        wk = w_qkv[C + g * DH: C + (g + 1) * DH]
        wv = w_qkv[2 * C + g * DH: 2 * C + (g + 1) * DH]
        wpT = w_proj[:, g * DH:(g + 1) * DH].T
        kbias = np.zeros((Mp,), np.float32)
        kbias[M:] = NEG
        xw = np.concatenate([xg.T, wq.T, wk.T, wv.T], axis=1)
        in_maps.append({
            "xw": np.ascontiguousarray(xw).astype(bf16),
            "wpT": np.ascontiguousarray(wpT).astype(bf16),
            "kbias": np.ascontiguousarray(kbias.reshape(-1, 128).T),
        })
    return in_maps, idxs


_NC_CACHE = {}


def kernel(**inputs) -> np.ndarray:
    x = np.asarray(inputs["x"], np.float32)
    mask = np.asarray(inputs["mask"], np.float32)
    w_qkv = np.asarray(inputs["w_qkv"], np.float32)
    w_proj = np.asarray(inputs["w_proj"], np.float32)

    m = mask[:, :, 0]
    Ms = [int(np.count_nonzero(m[b])) for b in range(B)]
    Mq = max(128, max(Ms))
    Mp = ((Mq + 127) // 128) * 128

    if (Mp, Mq) not in _NC_CACHE:
        _NC_CACHE[(Mp, Mq)] = build_nc(Mp, Mq)
    nc = _NC_CACHE[(Mp, Mq)]

    in_maps, idxs = _prep_core_inputs(x, mask, w_qkv, w_proj, Mp)
    res = run_bass_kernel_spmd(nc, in_maps, core_ids=list(range(8)))

    y_full = np.zeros((B, N, C), np.float32)
    for core in range(8):
        b = core // G
        idx = idxs[b]
        y_full[b, idx] += res.results[core]["y"][:len(idx)].astype(np.float32)
    return y_full
